# revision 7
# baseline (speedup 1.0000x reference)
"""Cost-volume layer (17-shift cross pattern, R=4) for Trainium2, 8 NeuronCores.

out[b,s,h,w] = sum_c src[b,c,h,w] * tgt[b,c,h+dh_s,w+dw_s]   (tgt zero-padded)

Strategy (column-progressive pipeline)
--------------------------------------
Shard: 8 cores = batch(4) x H-halves(2). Per core the inputs are HOST-
TRANSPOSED to w-major: src [C, W=160, 48], tgt [C, 160, 56] (8-row H halo,
W pad added on device via memset strips). C=128 is the SBUF partition dim,
contracted on the TensorEngine via banded correlations:

- vertical shifts  (dh=-4..4): per column w, matmul
    src[:, w, :]^T @ tgt[:, w+4, :]  ->  [48, 56] band
- horizontal shifts (dw=-4..4): per row h, per 32-col chunk i, matmul
    src[:, 32i:32i+32, h]^T @ tgt[:, 32i:32i+40, h+4] -> [32, 40] band

Because the layout is w-major, BOTH passes unlock column-piece by column-
piece: inputs load as 10 x 16-col pieces (tgt on the sync HWDGE ring, src
on the scalar ring, issued up front so the SDMA engines stream back to
back at full HBM rate), and the PE consumes V-bank(p) / H-bank(i) work the
moment piece p lands. The PE is never idle, which keeps the p-state ramp
at full clock. PSUM bands are staged to SBUF with full-partition-width
copies round-robined over DVE/ACT/Pool, and flushed to HBM on the sync
ring (which is idle after the input issues) so no compute sequencer ever
blocks on a DMA semaphore. Diagonals are gathered host-side from the
bands (pure indexing, no host arithmetic).
"""

import numpy as np
from contextlib import ExitStack

import concourse.bacc as bacc
import concourse.tile as tile
from concourse import mybir
from concourse import bass_utils

R = 4
B, C, H, W = 4, 128, 96, 160
NCORES = 8
HSH = H // 2            # 48 output rows per shard
HT = HSH + 2 * R        # 56 tgt rows (with halo)
WT = W + 2 * R          # 168 padded width (device)
F32 = mybir.dt.float32
F16 = mybir.dt.float16

PC = 16                 # piece width (cols)
NP = W // PC            # 10 pieces
# vertical: bank p covers w in [16p, 16p+16); w = 16p + 8g + s,
# g in {0,1} -> PSUM partition base 64g (holes 48:64), s in 0..8
VS = 8                  # slots per group
# horizontal: bank i covers chunks (h, i), h in 0..47; partition base
# 32*(h%4), slot h//4 in 0..11
MH = 32
NH = MH + 2 * R         # 40
NCH = W // MH           # 5 chunk columns = 5 H banks
HSLOT = HSH // 4        # 12

SHIFTS = [(0, 0)]
for i in range(1, R + 1):
    SHIFTS.extend([(-i, 0), (i, 0), (0, -i), (0, i)])


def build_nc():
    nc = bacc.Bacc("TRN2", target_bir_lowering=False)
    src = nc.dram_tensor("src", [C, W * HSH], F16, kind="ExternalInput")
    tgt = nc.dram_tensor("tgt", [C, W * HT], F16, kind="ExternalInput")
    # vband[gi] = V banks {2gi, 2gi+1}: [112, 2, 8, 56] (partition 64g+hh)
    vband = nc.dram_tensor("vband", [NP // 2, 112, 2 * VS * HT], F16,
                           kind="ExternalOutput")
    # hband[i] = H bank i: [128, 12, 40] (partition 32*(h%4)+m)
    hband = nc.dram_tensor("hband", [NCH, 128, HSLOT * NH], F16,
                           kind="ExternalOutput")

    with ExitStack() as ctx:
        tc = ctx.enter_context(tile.TileContext(nc))
        ins = ctx.enter_context(tc.tile_pool(name="ins", bufs=1))
        psum = ctx.enter_context(tc.tile_pool(name="psum", bufs=1, space="PSUM"))
        stage = ctx.enter_context(tc.tile_pool(name="stage", bufs=3))

        src_sb = ins.tile([C, W * HSH], F16)
        tgt_sb = ins.tile([C, WT * HT], F16)
        src3 = src_sb.rearrange("c (w h) -> c w h", h=HSH)
        tgt3 = tgt_sb.rearrange("c (w r) -> c w r", r=HT)
        srcd = src[:].rearrange("c (w h) -> c w h", h=HSH)
        tgtd = tgt[:].rearrange("c (w r) -> c w r", r=HT)

        # zero the W-pad strips (cols 0:4 and 164:168 of padded tgt)
        nc.gpsimd.memset(tgt3[:, 0:R, :], 0.0)
        nc.gpsimd.memset(tgt3[:, R + W:WT, :], 0.0)

        # PE p-state management: the TensorEngine only reaches full clock
        # (2.4GHz) after ~3us of CONTINUOUS execution; any idle gap drops it
        # back to 1.2GHz. Dummy matmuls on the (zeroed) pad strip keep the
        # pipeline hot: a pre-warm burst ramps the clock while the first
        # input piece is still in flight, and short filler bursts bridge the
        # per-piece DMA-semaphore gaps. Results go to a scratch PSUM bank
        # that is never read.
        warm = psum.tile([112, VS * HT], F32, tag="warm", bufs=1)

        def fill(n):
            for _ in range(n):
                nc.tensor.matmul(
                    out=warm[0:HSH, 0:HT],
                    lhsT=tgt3[:, 0, 0:HSH],
                    rhs=tgt3[:, 1, :],
                    start=True, stop=True,
                    tile_position=(0, 0),
                )

        # issue ALL input piece loads up front: tgt on sync, src on scalar.
        # SDMA streams them back-to-back; compute unlocks per piece via
        # Tile's subtile dependency tracking.
        for p in range(NP):
            nc.sync.dma_start(
                out=tgt3[:, R + PC * p:R + PC * (p + 1), :],
                in_=tgtd[:, PC * p:PC * (p + 1), :],
            )
        for p in range(NP):
            nc.scalar.dma_start(
                out=src3[:, PC * p:PC * (p + 1), :],
                in_=srcd[:, PC * p:PC * (p + 1), :],
            )

        copy_flip = [0]

        def stage_copy(dst, src_ap):
            # GPSIMD cannot access PSUM on TRN2 -> DVE/ACT only
            eng = (nc.vector.tensor_copy, nc.scalar.copy)[copy_flip[0] % 2]
            eng(out=dst, in_=src_ap)
            copy_flip[0] += 1

        vseg = 2 * VS * HT      # 896

        def vert_bank(p, st, half):
            pt = psum.tile([112, VS * HT], F32, tag="vp", bufs=3)
            for g in range(2):
                for s in range(VS):
                    w = PC * p + VS * g + s
                    nc.tensor.matmul(
                        out=pt[64 * g:64 * g + HSH, s * HT:(s + 1) * HT],
                        lhsT=src3[:, w, :],
                        rhs=tgt3[:, w + R, :],
                        start=True, stop=True,
                        tile_position=(0, 64 * g),
                    )
            # copy both groups (partitions 0:112; 48:64 are unwritten holes
            # the host ignores) in one full-width instruction
            stage_copy(st[:, half * (VS * HT):(half + 1) * (VS * HT)], pt)

        def horiz_bank(i, st):
            pt = psum.tile([128, HSLOT * NH], F32, tag="hp", bufs=2)
            for h in range(HSH):
                base = 32 * (h % 4)
                j = h // 4
                nc.tensor.matmul(
                    out=pt[base:base + MH, j * NH:(j + 1) * NH],
                    lhsT=src3[:, MH * i:MH * (i + 1), h],
                    rhs=tgt3[:, MH * i:MH * i + NH, h + R],
                    start=True, stop=True,
                    tile_position=(0, base),
                )
            stage_copy(st, pt)

        # schedule: V banks as pieces land; H bank i after piece 2i+2
        # (it needs tgt cols [32i, 32i+40) = pieces up to 2i+2 plus strips)
        fill(80)        # pre-warm: ~3.3us of PE busy before piece 0 lands
        vst = None
        hdone = 0
        for p in range(NP):
            if p > 0:
                fill(14)    # bridge the inter-piece semaphore gap
            if p % 2 == 0:
                vst = stage.tile([112, vseg], F16, tag="vs", name=f"vst{p}")
            vert_bank(p, vst, p % 2)
            if p % 2 == 1:
                nc.sync.dma_start(out=vband[:][p // 2], in_=vst)
            # H banks interleave: after piece 2,4,6,8 and the last piece
            while hdone < NCH and (p >= 2 * hdone + 2 or p == NP - 1):
                hst = stage.tile([128, HSLOT * NH], F16, tag="hs",
                                 name=f"hst{hdone}")
                horiz_bank(hdone, hst)
                nc.sync.dma_start(out=hband[:][hdone], in_=hst)
                hdone += 1

    nc.compile()
    return nc


_NC_CACHE = []


def _get_nc():
    if not _NC_CACHE:
        _NC_CACHE.append(build_nc())
    return _NC_CACHE[0]


def shard_inputs(src, tgt):
    src = np.asarray(src, dtype=np.float32)
    tgt = np.asarray(tgt, dtype=np.float32)
    tp = np.pad(tgt, ((0, 0), (0, 0), (R, R), (0, 0)))  # pad H only
    in_maps = []
    for core in range(NCORES):
        b, hh = divmod(core, 2)
        h0 = hh * HSH
        s = src[b, :, h0:h0 + HSH, :].transpose(0, 2, 1)       # [C, W, 48]
        t = tp[b, :, h0:h0 + HT, :].transpose(0, 2, 1)         # [C, W, 56]
        in_maps.append({
            "src": np.ascontiguousarray(s).reshape(C, W * HSH).astype(np.float16),
            "tgt": np.ascontiguousarray(t).reshape(C, W * HT).astype(np.float16),
        })
    return in_maps


def extract_output(results):
    """results: per core 'vband' [5, 112, 896], 'hband' [5, 128, 480]."""
    out = np.zeros((B, len(SHIFTS), H, W), np.float32)
    hidx = np.arange(HSH)
    midx = np.arange(MH)
    for core in range(NCORES):
        b, hh = divmod(core, 2)
        h0 = hh * HSH
        vb = np.asarray(results[core]["vband"]).astype(np.float32)
        vb = vb.reshape(NP // 2, 112, 2, VS, HT)   # [gi, part, half, s, r]
        # part = 64g + hh' (holes 48:64); w = 16*(2gi+half) + 8g + s
        vbg = np.stack([vb[:, 0:HSH], vb[:, 64:64 + HSH]], axis=1)
        # [gi, g, hh', half, s, r] -> [gi, half, g, s, hh', r] -> [w, hh', r]
        vbw = vbg.transpose(0, 3, 1, 4, 2, 5).reshape(W, HSH, HT)
        hb = np.asarray(results[core]["hband"]).astype(np.float32)
        hb = hb.reshape(NCH, 4, MH, HSLOT, NH)     # [i, h%4, m, h//4, n]
        hb = hb.transpose(3, 1, 0, 2, 4).reshape(HSH, NCH, MH, NH)
        for si, (dh, dw) in enumerate(SHIFTS):
            if dw == 0:
                v = vbw[:, hidx, hidx + dh + R]        # [W, 48]
                out[b, si, h0:h0 + HSH, :] = v.T
            else:
                v = hb[:, :, midx, midx + dw + R]      # [48, 5, 32]
                out[b, si, h0:h0 + HSH, :] = v.reshape(HSH, W)
    return out


def kernel(src, tgt, **run_kwargs):
    nc = _get_nc()
    in_maps = shard_inputs(src, tgt)
    res = bass_utils.run_bass_kernel_spmd(
        nc, in_maps, core_ids=list(range(NCORES)), **run_kwargs
    )
    out = extract_output(res.results)
    kernel.last_result = res
    return out


# revision 12
# speedup vs baseline: 1.0623x; 1.0623x over previous
"""Cost-volume layer (17-shift cross pattern, R=4) for Trainium2, 8 NeuronCores.

out[b,s,h,w] = sum_c src[b,c,h,w] * tgt[b,c,h+dh_s,w+dw_s]   (tgt zero-padded)

Strategy (column-progressive pipeline)
--------------------------------------
Shard: 8 cores = batch(4) x H-halves(2). Per core the inputs are HOST-
TRANSPOSED to w-major: src [C, W=160, 48], tgt [C, 160, 56] (8-row H halo,
W pad added on device via memset strips). C=128 is the SBUF partition dim,
contracted on the TensorEngine via banded correlations:

- vertical shifts  (dh=-4..4): per column w, matmul
    src[:, w, :]^T @ tgt[:, w+4, :]  ->  [48, 56] band
- horizontal shifts (dw=-4..4): per row h, per 32-col chunk i, matmul
    src[:, 32i:32i+32, h]^T @ tgt[:, 32i:32i+40, h+4] -> [32, 40] band

Because the layout is w-major, BOTH passes unlock column-piece by column-
piece: inputs load as 10 x 16-col pieces (tgt on the sync HWDGE ring, src
on the scalar ring, issued up front so the SDMA engines stream back to
back at full HBM rate), and the PE consumes V-bank(p) / H-bank(i) work the
moment piece p lands. The PE is never idle, which keeps the p-state ramp
at full clock. PSUM bands are staged to SBUF with full-partition-width
copies round-robined over DVE/ACT/Pool, and flushed to HBM on the sync
ring (which is idle after the input issues) so no compute sequencer ever
blocks on a DMA semaphore. Diagonals are gathered host-side from the
bands (pure indexing, no host arithmetic).
"""

import numpy as np
from contextlib import ExitStack

import concourse.bacc as bacc
import concourse.tile as tile
from concourse import mybir
from concourse import bass_utils

R = 4
B, C, H, W = 4, 128, 96, 160
NCORES = 8
HSH = H // 2            # 48 output rows per shard
HT = HSH + 2 * R        # 56 tgt rows (with halo)
WT = W + 2 * R          # 168 padded width (device)
F32 = mybir.dt.float32
F16 = mybir.dt.float16

PC = 16                 # piece width (cols)
NP = W // PC            # 10 pieces
# vertical: bank p covers w in [16p, 16p+16); w = 16p + 8g + s,
# g in {0,1} -> PSUM partition base 64g (holes 48:64), s in 0..8
VS = 8                  # slots per group
# horizontal: bank i covers chunks (h, i), h in 0..47; partition base
# 32*(h%4), slot h//4 in 0..11
MH = 32
NH = MH + 2 * R         # 40
NCH = W // MH           # 5 chunk columns = 5 H banks
HSLOT = HSH // 4        # 12

SHIFTS = [(0, 0)]
for i in range(1, R + 1):
    SHIFTS.extend([(-i, 0), (i, 0), (0, -i), (0, i)])


def build_nc():
    nc = bacc.Bacc("TRN2", target_bir_lowering=False)
    src = nc.dram_tensor("src", [C, W * HSH], F16, kind="ExternalInput")
    tgt = nc.dram_tensor("tgt", [C, W * HT], F16, kind="ExternalInput")
    # vband[p] = V bank p: [112, 8, 56] (partition 64g+hh, w = 16p+8g+s)
    vband = nc.dram_tensor("vband", [NP, 112, VS * HT], F16,
                           kind="ExternalOutput")
    # hband[i] = H bank i: [128, 12, 40] (partition 32*(h%4)+m)
    hband = nc.dram_tensor("hband", [NCH, 128, HSLOT * NH], F16,
                           kind="ExternalOutput")

    with ExitStack() as ctx:
        tc = ctx.enter_context(tile.TileContext(nc))
        ins = ctx.enter_context(tc.tile_pool(name="ins", bufs=1))
        psum = ctx.enter_context(tc.tile_pool(name="psum", bufs=1, space="PSUM"))
        stage = ctx.enter_context(tc.tile_pool(name="stage", bufs=3))

        src_sb = ins.tile([C, W * HSH], F16)
        tgt_sb = ins.tile([C, WT * HT], F16)
        src3 = src_sb.rearrange("c (w h) -> c w h", h=HSH)
        tgt3 = tgt_sb.rearrange("c (w r) -> c w r", r=HT)
        srcd = src[:].rearrange("c (w h) -> c w h", h=HSH)
        tgtd = tgt[:].rearrange("c (w r) -> c w r", r=HT)

        # zero the W-pad strips (cols 0:4 and 164:168 of padded tgt)
        nc.gpsimd.memset(tgt3[:, 0:R, :], 0.0)
        nc.gpsimd.memset(tgt3[:, R + W:WT, :], 0.0)

        # issue ALL input piece loads up front: tgt on sync, src on scalar.
        # SDMA streams them back-to-back; compute unlocks per piece via
        # Tile's subtile dependency tracking. The first 16-col piece is
        # split in two so the very first matmuls unlock ~1.5us earlier.
        cuts = [0, 8] + [PC * p for p in range(1, NP + 1)]
        for c0, c1 in zip(cuts[:-1], cuts[1:]):
            nc.sync.dma_start(
                out=tgt3[:, R + c0:R + c1, :],
                in_=tgtd[:, c0:c1, :],
            )
        for c0, c1 in zip(cuts[:-1], cuts[1:]):
            nc.scalar.dma_start(
                out=src3[:, c0:c1, :],
                in_=srcd[:, c0:c1, :],
            )

        copy_flip = [0]

        def stage_copy(dst, src_ap, eng=None):
            # GPSIMD cannot access PSUM on TRN2 -> DVE/ACT only
            if eng is None:
                eng = copy_flip[0] % 2
                copy_flip[0] += 1
            (nc.vector.tensor_copy, nc.scalar.copy)[eng](out=dst, in_=src_ap)

        def vert_bank(p, st, eng=None):
            pt = psum.tile([112, VS * HT], F32, tag="vp", bufs=3)
            for g in range(2):
                for s in range(VS):
                    w = PC * p + VS * g + s
                    nc.tensor.matmul(
                        out=pt[64 * g:64 * g + HSH, s * HT:(s + 1) * HT],
                        lhsT=src3[:, w, :],
                        rhs=tgt3[:, w + R, :],
                        start=True, stop=True,
                        tile_position=(0, 64 * g),
                    )
            # copy both groups (partitions 0:112; 48:64 are unwritten holes
            # the host ignores) in one full-width instruction
            stage_copy(st, pt, eng)

        def horiz_bank(i, st, eng=None):
            pt = psum.tile([128, HSLOT * NH], F32, tag="hp", bufs=2)
            for h in range(HSH):
                base = 32 * (h % 4)
                j = h // 4
                nc.tensor.matmul(
                    out=pt[base:base + MH, j * NH:(j + 1) * NH],
                    lhsT=src3[:, MH * i:MH * (i + 1), h],
                    rhs=tgt3[:, MH * i:MH * i + NH, h + R],
                    start=True, stop=True,
                    tile_position=(0, base),
                )
            stage_copy(st, pt, eng)

        def do_h(hdone, eng=None):
            hst = stage.tile([128, HSLOT * NH], F16, tag="hs",
                             name=f"hst{hdone}")
            horiz_bank(hdone, hst, eng)
            nc.sync.dma_start(out=hband[:][hdone], in_=hst)

        # schedule: V banks as pieces land; H bank i after piece 2i+2
        # (it needs tgt cols [32i, 32i+40) = pieces up to 2i+2 plus strips).
        # On the last piece run the (big) H bank first so its copy+flush
        # overlap the final V bank's matmuls; pin their copies to different
        # engines so they drain in parallel.
        hdone = 0
        for p in range(NP):
            last = p == NP - 1
            if last:
                while hdone < NCH:
                    do_h(hdone, eng=1)
                    hdone += 1
            vst = stage.tile([112, VS * HT], F16, tag="vs", name=f"vst{p}")
            vert_bank(p, vst, eng=0 if last else None)
            nc.sync.dma_start(out=vband[:][p], in_=vst)
            while hdone < NCH and p >= 2 * hdone + 2:
                do_h(hdone)
                hdone += 1

    nc.compile()
    return nc


_NC_CACHE = []


def _get_nc():
    if not _NC_CACHE:
        _NC_CACHE.append(build_nc())
    return _NC_CACHE[0]


def shard_inputs(src, tgt):
    src = np.asarray(src, dtype=np.float32)
    tgt = np.asarray(tgt, dtype=np.float32)
    tp = np.pad(tgt, ((0, 0), (0, 0), (R, R), (0, 0)))  # pad H only
    in_maps = []
    for core in range(NCORES):
        b, hh = divmod(core, 2)
        h0 = hh * HSH
        s = src[b, :, h0:h0 + HSH, :].transpose(0, 2, 1)       # [C, W, 48]
        t = tp[b, :, h0:h0 + HT, :].transpose(0, 2, 1)         # [C, W, 56]
        in_maps.append({
            "src": np.ascontiguousarray(s).reshape(C, W * HSH).astype(np.float16),
            "tgt": np.ascontiguousarray(t).reshape(C, W * HT).astype(np.float16),
        })
    return in_maps


def extract_output(results):
    """results: per core 'vband' [10, 112, 448], 'hband' [5, 128, 480]."""
    out = np.zeros((B, len(SHIFTS), H, W), np.float32)
    hidx = np.arange(HSH)
    midx = np.arange(MH)
    for core in range(NCORES):
        b, hh = divmod(core, 2)
        h0 = hh * HSH
        vb = np.asarray(results[core]["vband"]).astype(np.float32)
        vb = vb.reshape(NP, 112, VS, HT)           # [p, part, s, r]
        # part = 64g + hh' (holes 48:64); w = 16p + 8g + s
        vbg = np.stack([vb[:, 0:HSH], vb[:, 64:64 + HSH]], axis=1)
        # [p, g, hh', s, r] -> [p, g, s, hh', r] -> [w, hh', r]
        vbw = vbg.transpose(0, 1, 3, 2, 4).reshape(W, HSH, HT)
        hb = np.asarray(results[core]["hband"]).astype(np.float32)
        hb = hb.reshape(NCH, 4, MH, HSLOT, NH)     # [i, h%4, m, h//4, n]
        hb = hb.transpose(3, 1, 0, 2, 4).reshape(HSH, NCH, MH, NH)
        for si, (dh, dw) in enumerate(SHIFTS):
            if dw == 0:
                v = vbw[:, hidx, hidx + dh + R]        # [W, 48]
                out[b, si, h0:h0 + HSH, :] = v.T
            else:
                v = hb[:, :, midx, midx + dw + R]      # [48, 5, 32]
                out[b, si, h0:h0 + HSH, :] = v.reshape(HSH, W)
    return out


def kernel(src, tgt, **run_kwargs):
    nc = _get_nc()
    in_maps = shard_inputs(src, tgt)
    res = bass_utils.run_bass_kernel_spmd(
        nc, in_maps, core_ids=list(range(NCORES)), **run_kwargs
    )
    out = extract_output(res.results)
    kernel.last_result = res
    return out


# revision 17
# speedup vs baseline: 1.0971x; 1.0328x over previous
"""Cost-volume layer (17-shift cross pattern, R=4) for Trainium2, 8 NeuronCores.

out[b,s,h,w] = sum_c src[b,c,h,w] * tgt[b,c,h+dh_s,w+dw_s]   (tgt zero-padded)

Strategy (column-progressive pipeline)
--------------------------------------
Shard: 8 cores = batch(4) x H-halves(2). Per core the inputs are HOST-
TRANSPOSED to w-major: src [C, W=160, 48], tgt [C, 160, 56] (8-row H halo,
W pad added on device via memset strips). C=128 is the SBUF partition dim,
contracted on the TensorEngine via banded correlations:

- vertical shifts  (dh=-4..4): per column w, matmul
    src[:, w, :]^T @ tgt[:, w+4, :]  ->  [48, 56] band
- horizontal shifts (dw=-4..4): per row h, per 32-col chunk i, matmul
    src[:, 32i:32i+32, h]^T @ tgt[:, 32i:32i+40, h+4] -> [32, 40] band

Because the layout is w-major, BOTH passes unlock column-piece by column-
piece: inputs load as 10 x 16-col pieces (tgt on the sync HWDGE ring, src
on the scalar ring, issued up front so the SDMA engines stream back to
back at full HBM rate), and the PE consumes V-bank(p) / H-bank(i) work the
moment piece p lands. The PE is never idle, which keeps the p-state ramp
at full clock. PSUM bands are staged to SBUF with full-partition-width
copies round-robined over DVE/ACT/Pool, and flushed to HBM on the sync
ring (which is idle after the input issues) so no compute sequencer ever
blocks on a DMA semaphore. Diagonals are gathered host-side from the
bands (pure indexing, no host arithmetic).
"""

import numpy as np
from contextlib import ExitStack

import concourse.bacc as bacc
import concourse.tile as tile
from concourse import mybir
from concourse import bass_utils

R = 4
B, C, H, W = 4, 128, 96, 160
NCORES = 8
HSH = H // 2            # 48 output rows per shard
HT = HSH + 2 * R        # 56 tgt rows (with halo)
WT = W + 2 * R          # 168 padded width (device)
F32 = mybir.dt.float32
F16 = mybir.dt.float16

PC = 16                 # piece width (cols)
NP = W // PC            # 10 pieces
# vertical: bank p covers w in [16p, 16p+16); w = 16p + 8g + s,
# g in {0,1} -> PSUM partition base 64g (holes 48:64), s in 0..8
VS = 8                  # slots per group
# horizontal: bank i covers chunks (h, i), h in 0..47; partition base
# 32*(h%4), slot h//4 in 0..11
MH = 32
NH = MH + 2 * R         # 40
NCH = W // MH           # 5 chunk columns = 5 H banks
HSLOT = HSH // 4        # 12

SHIFTS = [(0, 0)]
for i in range(1, R + 1):
    SHIFTS.extend([(-i, 0), (i, 0), (0, -i), (0, i)])


def build_nc():
    nc = bacc.Bacc("TRN2", target_bir_lowering=False)
    src = nc.dram_tensor("src", [C, W * HSH], F16, kind="ExternalInput")
    tgt = nc.dram_tensor("tgt", [C, W * HT], F16, kind="ExternalInput")
    # vband[gi] = V banks {2gi,2gi+1}: [112, 2, 8, 56] (partition 64g+hh)
    vband = nc.dram_tensor("vband", [NP // 2, 112, 2 * VS * HT], F16,
                           kind="ExternalOutput")
    # hband[i] = H bank i: [128, 12, 40] (partition 32*(h%4)+m)
    hband = nc.dram_tensor("hband", [NCH, 128, HSLOT * NH], F16,
                           kind="ExternalOutput")

    with ExitStack() as ctx:
        tc = ctx.enter_context(tile.TileContext(nc))
        ins = ctx.enter_context(tc.tile_pool(name="ins", bufs=1))
        psum = ctx.enter_context(tc.tile_pool(name="psum", bufs=1, space="PSUM"))
        stage = ctx.enter_context(tc.tile_pool(name="stage", bufs=3))

        src_sb = ins.tile([C, W * HSH], F16)
        tgt_sb = ins.tile([C, WT * HT], F16)
        src3 = src_sb.rearrange("c (w h) -> c w h", h=HSH)
        tgt3 = tgt_sb.rearrange("c (w r) -> c w r", r=HT)
        srcd = src[:].rearrange("c (w h) -> c w h", h=HSH)
        tgtd = tgt[:].rearrange("c (w r) -> c w r", r=HT)

        # zero the W-pad strips (cols 0:4 and 164:168 of padded tgt)
        nc.gpsimd.memset(tgt3[:, 0:R, :], 0.0)
        nc.gpsimd.memset(tgt3[:, R + W:WT, :], 0.0)

        # issue ALL input piece loads up front: tgt on sync, src on scalar.
        # SDMA streams them back-to-back; compute unlocks per piece via
        # Tile's subtile dependency tracking. The first 16-col piece is
        # split in two so the very first matmuls unlock ~1.5us earlier.
        cuts = [0, 8] + [PC * p for p in range(1, NP + 1)]
        for c0, c1 in zip(cuts[:-1], cuts[1:]):
            nc.sync.dma_start(
                out=tgt3[:, R + c0:R + c1, :],
                in_=tgtd[:, c0:c1, :],
            )
        for c0, c1 in zip(cuts[:-1], cuts[1:]):
            nc.scalar.dma_start(
                out=src3[:, c0:c1, :],
                in_=srcd[:, c0:c1, :],
            )

        copy_flip = [0]

        def stage_copy(dst, src_ap, eng=None):
            # GPSIMD cannot access PSUM on TRN2 -> DVE/ACT only
            if eng is None:
                eng = copy_flip[0] % 2
                copy_flip[0] += 1
            (nc.vector.tensor_copy, nc.scalar.copy)[eng](out=dst, in_=src_ap)

        def vert_bank(p, st, half, eng=None):
            pt = psum.tile([112, VS * HT], F32, tag="vp", bufs=3)
            for g in range(2):
                for s in range(VS):
                    w = PC * p + VS * g + s
                    nc.tensor.matmul(
                        out=pt[64 * g:64 * g + HSH, s * HT:(s + 1) * HT],
                        lhsT=src3[:, w, :],
                        rhs=tgt3[:, w + R, :],
                        start=True, stop=True,
                        tile_position=(0, 64 * g),
                    )
            # copy both groups (partitions 0:112; 48:64 are unwritten holes
            # the host ignores) in one full-width instruction
            seg = VS * HT
            stage_copy(st[:, half * seg:(half + 1) * seg], pt, eng)

        def horiz_bank(i, st, eng=None):
            pt = psum.tile([128, HSLOT * NH], F32, tag="hp", bufs=2)
            for h in range(HSH):
                base = 32 * (h % 4)
                j = h // 4
                nc.tensor.matmul(
                    out=pt[base:base + MH, j * NH:(j + 1) * NH],
                    lhsT=src3[:, MH * i:MH * (i + 1), h],
                    rhs=tgt3[:, MH * i:MH * i + NH, h + R],
                    start=True, stop=True,
                    tile_position=(0, base),
                )
            stage_copy(st, pt, eng)

        def do_h(hdone, eng=None):
            hst = stage.tile([128, HSLOT * NH], F16, tag="hs", bufs=4,
                             name=f"hst{hdone}")
            horiz_bank(hdone, hst, eng)
            # flushes ride the GPSIMD SWDGE queue: ~25ns sequencer dispatch
            # and async Q7 descriptor generation, so they never contend with
            # the input loads on the HWDGE rings
            nc.gpsimd.dma_start(out=hband[:][hdone], in_=hst)

        # schedule: V banks as pieces land; H bank i after piece 2i+2
        # (it needs tgt cols [32i, 32i+40) = pieces up to 2i+2 plus strips).
        # On the last piece run the (big) H bank first so its copy+flush
        # overlap the final V bank's matmuls; pin their copies to different
        # engines so they drain in parallel.
        vst = None
        hdone = 0
        for p in range(NP):
            last = p == NP - 1
            if last:
                while hdone < NCH:
                    do_h(hdone, eng=1)
                    hdone += 1
            if p % 2 == 0:
                vst = stage.tile([112, 2 * VS * HT], F16, tag="vs", bufs=4,
                                 name=f"vst{p}")
            vert_bank(p, vst, p % 2, eng=0 if last else None)
            if p % 2 == 1:
                nc.gpsimd.dma_start(out=vband[:][p // 2], in_=vst)

    nc.compile()
    return nc


_NC_CACHE = []


def _get_nc():
    if not _NC_CACHE:
        _NC_CACHE.append(build_nc())
    return _NC_CACHE[0]


def shard_inputs(src, tgt):
    src = np.asarray(src, dtype=np.float32)
    tgt = np.asarray(tgt, dtype=np.float32)
    tp = np.pad(tgt, ((0, 0), (0, 0), (R, R), (0, 0)))  # pad H only
    in_maps = []
    for core in range(NCORES):
        b, hh = divmod(core, 2)
        h0 = hh * HSH
        s = src[b, :, h0:h0 + HSH, :].transpose(0, 2, 1)       # [C, W, 48]
        t = tp[b, :, h0:h0 + HT, :].transpose(0, 2, 1)         # [C, W, 56]
        in_maps.append({
            "src": np.ascontiguousarray(s).reshape(C, W * HSH).astype(np.float16),
            "tgt": np.ascontiguousarray(t).reshape(C, W * HT).astype(np.float16),
        })
    return in_maps


def extract_output(results):
    """results: per core 'vband' [5, 112, 896], 'hband' [5, 128, 480]."""
    out = np.zeros((B, len(SHIFTS), H, W), np.float32)
    hidx = np.arange(HSH)
    midx = np.arange(MH)
    for core in range(NCORES):
        b, hh = divmod(core, 2)
        h0 = hh * HSH
        vb = np.asarray(results[core]["vband"]).astype(np.float32)
        vb = vb.reshape(NP // 2, 112, 2, VS, HT)   # [gi, part, half, s, r]
        # part = 64g + hh' (holes 48:64); w = 16*(2gi+half) + 8g + s
        vbg = np.stack([vb[:, 0:HSH], vb[:, 64:64 + HSH]], axis=1)
        # [gi, g, hh', half, s, r] -> [gi, half, g, s, hh', r]
        vbw = vbg.transpose(0, 3, 1, 4, 2, 5).reshape(W, HSH, HT)
        hb = np.asarray(results[core]["hband"]).astype(np.float32)
        hb = hb.reshape(NCH, 4, MH, HSLOT, NH)     # [i, h%4, m, h//4, n]
        hb = hb.transpose(3, 1, 0, 2, 4).reshape(HSH, NCH, MH, NH)
        for si, (dh, dw) in enumerate(SHIFTS):
            if dw == 0:
                v = vbw[:, hidx, hidx + dh + R]        # [W, 48]
                out[b, si, h0:h0 + HSH, :] = v.T
            else:
                v = hb[:, :, midx, midx + dw + R]      # [48, 5, 32]
                out[b, si, h0:h0 + HSH, :] = v.reshape(HSH, W)
    return out


def kernel(src, tgt, **run_kwargs):
    nc = _get_nc()
    in_maps = shard_inputs(src, tgt)
    res = bass_utils.run_bass_kernel_spmd(
        nc, in_maps, core_ids=list(range(NCORES)), **run_kwargs
    )
    out = extract_output(res.results)
    kernel.last_result = res
    return out
